# revision 35
# baseline (speedup 1.0000x reference)
"""Bandpass biquad (torchaudio bandpass_biquad, const_skirt_gain=False) on 8 TRN2 cores.

Approach: the IIR filter's poles have radius ~0.83, so the impulse response
decays below 3e-11 by lag 128. Truncate to a 128-tap FIR and compute it as
banded-Toeplitz-weight matmuls on TensorEngine (bf16 operands, fp32 PSUM
accumulate; set IN_BF16/OUT_BF16 = False for a float32r/f32 variant with
~1.4e-4 rel err at ~1.8x the runtime).

Sharding: time dimension split across the 8 cores (25088 output samples each,
plus 128 samples of history); all 128 batch rows live on the SBUF partition
dim on every core. The host pre-transposes the input into per-block
[time-within-block, batch] layout so all DMAs are large and contiguous.

Per core, per 512-sample output tile i: out[b, f] = sum_{d=0}^{4} sum_p
  X[128*i*4 + 128*d + p, b] * W_d[p, f],  W_d[p, f] = h[f + 128 - 128*d - p]
accumulated over 5 matmuls in one PSUM bank (out = lhsT.T @ rhs with
lhsT = the x block [p, b] as the stationary operand, rhs = the constant
Toeplitz band [p, f] as the moving operand). Matmul d=0 is full-width so
its start=True clears has_written for the whole bank (HW clears the whole
bank, verified); d=1..4 cover only their nonzero bands.

Pipeline per core: inputs on the Sync HWDGE ring (7x 875KB), weights +
outputs on the Scalar HWDGE ring (3 output chunks per group to bound
sequencer issue cost at ~0.7us/DMA), PSUM->SBUF bf16 cast-copies on DVE,
8 PSUM banks / 8 x-tile / 6 y-tile buffers for deep pipelining.
"""

import math

import numpy as np

# ---- problem constants (hardcoded; kernel.py must be self-contained) ----
SR = 48000.0
CENTRAL_FREQ = 2000.0
Q = 0.707

B = 128            # batch rows
T = 200000         # samples per row
NCORES = 8
TILE = 512         # output samples per PSUM tile
NBLK = 5           # x blocks (of 128) contributing to one 512 tile
L = 128            # FIR taps
T_CORE = 25088     # output samples per core (= 49 * 512)
G = 7              # DMA groups per core
S = 7              # PSUM tiles per group
JBLK = 28          # 128-sample input blocks per group (= 4 * S)
T_PAD = NCORES * T_CORE  # 200704


def _coeffs_f32():
    w0 = 2.0 * math.pi * CENTRAL_FREQ / SR
    alpha = math.sin(w0) / (2.0 * Q)
    b0, b1, b2 = alpha, 0.0, -alpha
    a0, a1, a2 = 1.0 + alpha, -2.0 * math.cos(w0), 1.0 - alpha
    # the reference rounds each normalized coefficient to float32
    return tuple(np.float32(c / a0) for c in (b0, b1, b2, a1, a2))


def _impulse_response(n=L):
    b0, b1, b2, a1, a2 = [float(c) for c in _coeffs_f32()]
    h = np.zeros(n, dtype=np.float64)
    x1 = x2 = y1 = y2 = 0.0
    for i in range(n):
        xn = 1.0 if i == 0 else 0.0
        yn = b0 * xn + b1 * x1 + b2 * x2 - a1 * y1 - a2 * y2
        h[i] = yn
        x2, x1 = x1, xn
        y2, y1 = y1, yn
    return h


def _toeplitz_weights(np_dtype=np.float32, e4=256):
    """Banded Toeplitz weights, columns = concat of 5 segments.

    Segment d covers out-range [f0_d, f0_d+width_d):
      d=0: [0,512) full (start=True clears the whole PSUM tile)
      d=1: [0,256), d=2: [128,384), d=3: [256,512), d=4: [512-e4,512)
    W_d[p, f] = h[f + 128 - 128*d - p].
    """
    h = _impulse_response()
    segs = _w_segments(e4)
    cols = []
    p = np.arange(128)[:, None]
    for f0, width, d in segs:
        f = f0 + np.arange(width)[None, :]
        k = f + 128 - 128 * d - p
        valid = (k >= 0) & (k < L)
        Wd = np.where(valid, h[np.clip(k, 0, L - 1)], 0.0)
        cols.append(Wd)
    W = np.concatenate(cols, axis=1)
    return np.ascontiguousarray(W).astype(np_dtype)


def _w_segments(e4):
    return [(0, 512, 0), (0, 256, 1), (128, 256, 2), (256, 256, 3),
            (512 - e4, e4, 4)]


IN_BF16 = True
OUT_BF16 = True


def build_bass():
    import concourse.mybir as mybir
    from concourse import bacc
    from concourse.tile import TileContext

    f32 = mybir.dt.float32
    in_dt = mybir.dt.bfloat16 if IN_BF16 else mybir.dt.float32r
    out_dt = mybir.dt.bfloat16 if OUT_BF16 else mybir.dt.float32
    e4 = 128 if IN_BF16 else 256
    segs = _w_segments(e4)
    wcols = sum(w for _, w, _ in segs)
    woff = np.cumsum([0] + [w for _, w, _ in segs])

    nc = bacc.Bacc(
        "TRN2",
        target_bir_lowering=False,
        debug=False,
        num_devices=NCORES,
    )
    x = nc.declare_dram_parameter("x", [G, 128, JBLK, 128], in_dt, isOutput=False)
    x0 = nc.declare_dram_parameter("x0", [128, 128], in_dt, isOutput=False)
    w = nc.declare_dram_parameter("w", [128, wcols], in_dt, isOutput=False)
    out = nc.declare_dram_parameter("out", [128, T_CORE], out_dt, isOutput=True)

    with TileContext(nc) as tc:
        with (
            tc.tile_pool(name="wp", bufs=1) as wp,
            tc.tile_pool(name="xp", bufs=8) as xp,
            tc.tile_pool(name="yp", bufs=6) as yp,
            tc.tile_pool(name="pp", bufs=8, space="PSUM") as pp,
        ):
            x0t = wp.tile([128, 128], in_dt)
            nc.sync.dma_start(out=x0t[:], in_=x0[:])
            wt = wp.tile([128, wcols], in_dt)
            nc.scalar.dma_start(out=wt[:], in_=w[:])

            # PE HAM warm-up: keep TensorE busy on throwaway matmuls while
            # the first input/weight DMAs are still in flight, so the clock
            # gate releases (1.2 -> 2.4 GHz) before the real matmuls start.
            warm_src = wp.tile([128, 128], in_dt)
            nc.gpsimd.memset(warm_src[:], 0.0)
            warm = pp.tile([128, TILE], f32, tag="pt")
            for _ in range(34):
                nc.tensor.matmul(
                    warm[:, :128], warm_src[:], warm_src[:], start=True, stop=True
                )

            prev_last = x0t[:]
            for g in range(G):
                xt = xp.tile([128, JBLK * 128], in_dt)
                nc.sync.dma_start(
                    out=xt[:], in_=x[g].rearrange("p j b -> p (j b)")
                )
                yt = yp.tile([128, S * TILE], out_dt)
                for s in range(S):
                    pt = pp.tile([128, TILE], f32)
                    for d in range(NBLK):
                        j = 4 * s + d - 1
                        lhsT = prev_last if j < 0 else xt[:, j * 128:(j + 1) * 128]
                        f0, width, _ = segs[d]
                        nc.tensor.matmul(
                            pt[:, f0:f0 + width],
                            lhsT,
                            wt[:, int(woff[d]):int(woff[d + 1])],
                            start=(d == 0),
                            stop=(d == NBLK - 1),
                        )
                    ysl = yt[:, s * TILE:(s + 1) * TILE]
                    nc.vector.tensor_copy(out=ysl, in_=pt[:])
                t0 = g * S * TILE
                for c0, c1 in ((0, 3), (3, 5), (5, 7)):
                    nc.scalar.dma_start(
                        out=out[:, t0 + c0 * TILE:t0 + c1 * TILE],
                        in_=yt[:, c0 * TILE:c1 * TILE],
                    )
                prev_last = xt[:, (JBLK - 1) * 128:JBLK * 128]
    nc.compile()
    return nc


def shard_inputs(wf):
    """wf: [128, 200000] f32 -> list of 8 in_maps."""
    if IN_BF16:
        import ml_dtypes
        np_in = ml_dtypes.bfloat16
    else:
        np_in = np.float32
    wmat = _toeplitz_weights(np_in, e4=128 if IN_BF16 else 256)
    X = np.zeros((B, 128 + T_PAD), dtype=np_in)
    X[:, 128:128 + T] = wf.astype(np_in)
    in_maps = []
    for c in range(NCORES):
        base = c * T_CORE
        sl = X[:, base:base + 128 + T_CORE]
        x0 = np.ascontiguousarray(sl[:, :128].T)
        body = sl[:, 128:]  # [b, T_CORE]
        xc = np.ascontiguousarray(
            body.reshape(B, G, JBLK, 128).transpose(1, 3, 2, 0)
        )
        in_maps.append({"x": xc, "x0": x0, "w": wmat})
    return in_maps


_CACHED = {}


def _run_with_retry(nc, in_maps):
    from concourse.bass_utils import run_bass_kernel_spmd

    last_err = None
    for attempt in range(3):
        try:
            return run_bass_kernel_spmd(nc, in_maps, core_ids=list(range(NCORES)))
        except Exception as e:  # transient device/runtime failures
            last_err = e
            try:
                import jax
                jax.clear_caches()
                jax.clear_backends()
            except Exception:
                pass
            import time
            time.sleep(3.0 * (attempt + 1))
    raise last_err


def kernel(waveform):
    wf = np.ascontiguousarray(np.asarray(waveform, dtype=np.float32))
    assert wf.shape == (B, T), wf.shape
    in_maps = shard_inputs(wf)
    if "nc" not in _CACHED:
        _CACHED["nc"] = build_bass()
    res = _run_with_retry(_CACHED["nc"], in_maps)
    y = np.concatenate(
        [np.asarray(res.results[c]["out"]) for c in range(NCORES)], axis=1
    )
    return np.ascontiguousarray(y[:, :T].astype(np.float32))


# revision 40
# speedup vs baseline: 1.0282x; 1.0282x over previous
"""Bandpass biquad (torchaudio bandpass_biquad, const_skirt_gain=False) on 8 TRN2 cores.

Approach: the IIR filter's poles have radius ~0.83, so the impulse response
decays below 3e-11 by lag 128. Truncate to a 128-tap FIR and compute it as
banded-Toeplitz-weight matmuls on TensorEngine (bf16 operands, fp32 PSUM
accumulate; set IN_BF16/OUT_BF16 = False for a float32r/f32 variant with
~1.4e-4 rel err at ~1.8x the runtime).

Sharding: time dimension split across the 8 cores (25088 output samples each,
plus 128 samples of history); all 128 batch rows live on the SBUF partition
dim on every core. The host pre-transposes the input into per-block
[time-within-block, batch] layout so all DMAs are large and contiguous.

Per core, per 512-sample output tile i: out[b, f] = sum_{d=0}^{4} sum_p
  X[128*i*4 + 128*d + p, b] * W_d[p, f],  W_d[p, f] = h[f + 128 - 128*d - p]
accumulated over 5 matmuls in one PSUM bank (out = lhsT.T @ rhs with
lhsT = the x block [p, b] as the stationary operand, rhs = the constant
Toeplitz band [p, f] as the moving operand). Matmul d=0 is full-width so
its start=True clears has_written for the whole bank (HW clears the whole
bank, verified); d=1..4 cover only their nonzero bands.

Pipeline per core: inputs on the Sync HWDGE ring (7x 875KB), weights +
outputs on the Scalar HWDGE ring (3 output chunks per group to bound
sequencer issue cost at ~0.7us/DMA), PSUM->SBUF bf16 cast-copies on DVE,
8 PSUM banks / 8 x-tile / 6 y-tile buffers for deep pipelining.
"""

import math

import numpy as np

# ---- problem constants (hardcoded; kernel.py must be self-contained) ----
SR = 48000.0
CENTRAL_FREQ = 2000.0
Q = 0.707

B = 128            # batch rows
T = 200000         # samples per row
NCORES = 8
TILE = 512         # output samples per PSUM tile
NBLK = 5           # x blocks (of 128) contributing to one 512 tile
L = 128            # FIR taps
T_CORE = 25088     # output samples per core (= 49 * 512)
G = 7              # DMA groups per core
S = 7              # PSUM tiles per group
JBLK = 28          # 128-sample input blocks per group (= 4 * S)
T_PAD = NCORES * T_CORE  # 200704


def _coeffs_f32():
    w0 = 2.0 * math.pi * CENTRAL_FREQ / SR
    alpha = math.sin(w0) / (2.0 * Q)
    b0, b1, b2 = alpha, 0.0, -alpha
    a0, a1, a2 = 1.0 + alpha, -2.0 * math.cos(w0), 1.0 - alpha
    # the reference rounds each normalized coefficient to float32
    return tuple(np.float32(c / a0) for c in (b0, b1, b2, a1, a2))


def _impulse_response(n=L):
    b0, b1, b2, a1, a2 = [float(c) for c in _coeffs_f32()]
    h = np.zeros(n, dtype=np.float64)
    x1 = x2 = y1 = y2 = 0.0
    for i in range(n):
        xn = 1.0 if i == 0 else 0.0
        yn = b0 * xn + b1 * x1 + b2 * x2 - a1 * y1 - a2 * y2
        h[i] = yn
        x2, x1 = x1, xn
        y2, y1 = y1, yn
    return h


def _toeplitz_weights(np_dtype=np.float32, e4=256):
    """Banded Toeplitz weights, columns = concat of 5 segments.

    Segment d covers out-range [f0_d, f0_d+width_d):
      d=0: [0,512) full (start=True clears the whole PSUM tile)
      d=1: [0,256), d=2: [128,384), d=3: [256,512), d=4: [512-e4,512)
    W_d[p, f] = h[f + 128 - 128*d - p].
    """
    h = _impulse_response()
    segs = _w_segments(e4)
    cols = []
    p = np.arange(128)[:, None]
    for f0, width, d in segs:
        f = f0 + np.arange(width)[None, :]
        k = f + 128 - 128 * d - p
        valid = (k >= 0) & (k < L)
        Wd = np.where(valid, h[np.clip(k, 0, L - 1)], 0.0)
        cols.append(Wd)
    W = np.concatenate(cols, axis=1)
    return np.ascontiguousarray(W).astype(np_dtype)


def _w_segments(e4):
    return [(0, 512, 0), (0, 256, 1), (128, 256, 2), (256, 256, 3),
            (512 - e4, e4, 4)]


IN_BF16 = True
OUT_BF16 = True


def build_bass():
    import concourse.mybir as mybir
    from concourse import bacc
    from concourse.tile import TileContext

    f32 = mybir.dt.float32
    in_dt = mybir.dt.bfloat16 if IN_BF16 else mybir.dt.float32r
    out_dt = mybir.dt.bfloat16 if OUT_BF16 else mybir.dt.float32
    e4 = 128 if IN_BF16 else 256
    segs = _w_segments(e4)
    wcols = sum(w for _, w, _ in segs)
    woff = np.cumsum([0] + [w for _, w, _ in segs])

    nc = bacc.Bacc(
        "TRN2",
        target_bir_lowering=False,
        debug=False,
        num_devices=NCORES,
    )
    x = nc.declare_dram_parameter("x", [G, 128, JBLK, 128], in_dt, isOutput=False)
    x0 = nc.declare_dram_parameter("x0", [128, 128], in_dt, isOutput=False)
    w = nc.declare_dram_parameter("w", [128, wcols], in_dt, isOutput=False)
    out = nc.declare_dram_parameter("out", [128, T_CORE], out_dt, isOutput=True)

    with TileContext(nc) as tc:
        with (
            tc.tile_pool(name="wp", bufs=1) as wp,
            tc.tile_pool(name="xp", bufs=8) as xp,
            tc.tile_pool(name="yp", bufs=6) as yp,
            tc.tile_pool(name="pp", bufs=8, space="PSUM") as pp,
        ):
            x0t = wp.tile([128, 128], in_dt)
            nc.sync.dma_start(out=x0t[:], in_=x0[:])
            wt = wp.tile([128, wcols], in_dt)
            nc.scalar.dma_start(out=wt[:], in_=w[:])

            # PE HAM warm-up: keep TensorE busy on throwaway matmuls while
            # the first input/weight DMAs are still in flight, so the clock
            # gate releases (1.2 -> 2.4 GHz) before the real matmuls start.
            warm_src = wp.tile([128, 128], in_dt)
            nc.gpsimd.memset(warm_src[:], 0.0)
            warm = pp.tile([128, TILE], f32, tag="pt")
            for _ in range(34):
                nc.tensor.matmul(
                    warm[:, :128], warm_src[:], warm_src[:], start=True, stop=True
                )

            prev_last = x0t[:]
            for g in range(G):
                xt = xp.tile([128, JBLK * 128], in_dt)
                nc.sync.dma_start(
                    out=xt[:], in_=x[g].rearrange("p j b -> p (j b)")
                )
                yt = yp.tile([128, S * TILE], out_dt)
                for s in range(S):
                    pt = pp.tile([128, TILE], f32)
                    for d in range(NBLK):
                        j = 4 * s + d - 1
                        lhsT = prev_last if j < 0 else xt[:, j * 128:(j + 1) * 128]
                        f0, width, _ = segs[d]
                        nc.tensor.matmul(
                            pt[:, f0:f0 + width],
                            lhsT,
                            wt[:, int(woff[d]):int(woff[d + 1])],
                            start=(d == 0),
                            stop=(d == NBLK - 1),
                        )
                    ysl = yt[:, s * TILE:(s + 1) * TILE]
                    nc.vector.tensor_copy(out=ysl, in_=pt[:])
                t0 = g * S * TILE
                for c0, c1 in ((0, 3), (3, 5), (5, 7)):
                    nc.scalar.dma_start(
                        out=out[:, t0 + c0 * TILE:t0 + c1 * TILE],
                        in_=yt[:, c0 * TILE:c1 * TILE],
                    )
                prev_last = xt[:, (JBLK - 1) * 128:JBLK * 128]
    nc.compile()
    return nc


def shard_inputs(wf):
    """wf: [128, 200000] f32 -> list of 8 in_maps."""
    if IN_BF16:
        import ml_dtypes
        np_in = ml_dtypes.bfloat16
    else:
        np_in = np.float32
    wmat = _toeplitz_weights(np_in, e4=128 if IN_BF16 else 256)
    X = np.zeros((B, 128 + T_PAD), dtype=np_in)
    X[:, 128:128 + T] = wf.astype(np_in)
    in_maps = []
    for c in range(NCORES):
        base = c * T_CORE
        sl = X[:, base:base + 128 + T_CORE]
        x0 = np.ascontiguousarray(sl[:, :128].T)
        body = sl[:, 128:]  # [b, T_CORE]
        xc = np.ascontiguousarray(
            body.reshape(B, G, JBLK, 128).transpose(1, 3, 2, 0)
        )
        in_maps.append({"x": xc, "x0": x0, "w": wmat})
    return in_maps


_CACHED = {}


def _run_with_retry(nc, in_maps):
    from concourse.bass_utils import run_bass_kernel_spmd

    last_err = None
    for attempt in range(3):
        try:
            return run_bass_kernel_spmd(nc, in_maps, core_ids=list(range(NCORES)))
        except Exception as e:  # transient device/runtime failures
            last_err = e
            try:
                import jax
                jax.clear_caches()
                jax.clear_backends()
            except Exception:
                pass
            import time
            time.sleep(3.0 * (attempt + 1))
    raise last_err


def kernel(waveform):
    wf = np.ascontiguousarray(np.asarray(waveform, dtype=np.float32))
    assert wf.shape == (B, T), wf.shape
    in_maps = shard_inputs(wf)
    if "nc" not in _CACHED:
        _CACHED["nc"] = build_bass()
    res = _run_with_retry(_CACHED["nc"], in_maps)
    y = np.concatenate(
        [np.asarray(res.results[c]["out"]) for c in range(NCORES)], axis=1
    )
    return np.ascontiguousarray(y[:, :T].astype(np.float32))


# revision 45
# speedup vs baseline: 1.1066x; 1.0762x over previous
"""Bandpass biquad (torchaudio bandpass_biquad, const_skirt_gain=False) on 8 TRN2 cores.

Approach: the IIR filter's poles have radius ~0.83, so the impulse response
decays below 3e-11 by lag 128. Truncate to a 128-tap FIR and compute it as
banded-Toeplitz-weight matmuls on TensorEngine (bf16 operands, fp32 PSUM
accumulate; set IN_BF16/OUT_BF16 = False for a float32r/f32 variant with
~1.4e-4 rel err at ~1.8x the runtime).

Sharding: time dimension split across the 8 cores (25088 output samples each,
plus 128 samples of history); all 128 batch rows live on the SBUF partition
dim on every core. The host pre-transposes the input into per-block
[time-within-block, batch] layout so all DMAs are large and contiguous.

Per core, per 512-sample output tile i: out[b, f] = sum_{d=0}^{4} sum_p
  X[128*i*4 + 128*d + p, b] * W_d[p, f],  W_d[p, f] = h[f + 128 - 128*d - p]
accumulated over 5 matmuls in one PSUM bank (out = lhsT.T @ rhs with
lhsT = the x block [p, b] as the stationary operand, rhs = the constant
Toeplitz band [p, f] as the moving operand). Matmul d=0 is full-width so
its start=True clears has_written for the whole bank (HW clears the whole
bank, verified); d=1..4 cover only their nonzero bands.

Pipeline per core: inputs on the Sync HWDGE ring (7x 875KB), weights +
outputs on the Scalar HWDGE ring (3 output chunks per group to bound
sequencer issue cost at ~0.7us/DMA), PSUM->SBUF bf16 cast-copies on DVE,
8 PSUM banks / 8 x-tile / 6 y-tile buffers for deep pipelining.
"""

import math

import numpy as np

# ---- problem constants (hardcoded; kernel.py must be self-contained) ----
SR = 48000.0
CENTRAL_FREQ = 2000.0
Q = 0.707

B = 128            # batch rows
T = 200000         # samples per row
NCORES = 8
TILE = 512         # output samples per PSUM tile
NBLK = 5           # x blocks (of 128) contributing to one 512 tile
L = 128            # FIR taps
T_CORE = 25088     # output samples per core (= 49 * 512)
G = 7              # DMA groups per core
S = 7              # PSUM tiles per group
JBLK = 28          # 128-sample input blocks per group (= 4 * S)
T_PAD = NCORES * T_CORE  # 200704


def _coeffs_f32():
    w0 = 2.0 * math.pi * CENTRAL_FREQ / SR
    alpha = math.sin(w0) / (2.0 * Q)
    b0, b1, b2 = alpha, 0.0, -alpha
    a0, a1, a2 = 1.0 + alpha, -2.0 * math.cos(w0), 1.0 - alpha
    # the reference rounds each normalized coefficient to float32
    return tuple(np.float32(c / a0) for c in (b0, b1, b2, a1, a2))


def _impulse_response(n=L):
    b0, b1, b2, a1, a2 = [float(c) for c in _coeffs_f32()]
    h = np.zeros(n, dtype=np.float64)
    x1 = x2 = y1 = y2 = 0.0
    for i in range(n):
        xn = 1.0 if i == 0 else 0.0
        yn = b0 * xn + b1 * x1 + b2 * x2 - a1 * y1 - a2 * y2
        h[i] = yn
        x2, x1 = x1, xn
        y2, y1 = y1, yn
    return h


def _toeplitz_weights(np_dtype=np.float32, e4=256):
    """Banded Toeplitz weights, columns = concat of 5 segments.

    Segment d covers out-range [f0_d, f0_d+width_d):
      d=0: [0,512) full (start=True clears the whole PSUM tile)
      d=1: [0,256), d=2: [128,384), d=3: [256,512), d=4: [512-e4,512)
    W_d[p, f] = h[f + 128 - 128*d - p].
    """
    h = _impulse_response()
    segs = _w_segments(e4)
    cols = []
    p = np.arange(128)[:, None]
    for f0, width, d in segs:
        f = f0 + np.arange(width)[None, :]
        k = f + 128 - 128 * d - p
        valid = (k >= 0) & (k < L)
        Wd = np.where(valid, h[np.clip(k, 0, L - 1)], 0.0)
        cols.append(Wd)
    W = np.concatenate(cols, axis=1)
    return np.ascontiguousarray(W).astype(np_dtype)


def _w_segments(e4):
    return [(0, 512, 0), (0, 256, 1), (128, 256, 2), (256, 256, 3),
            (512 - e4, e4, 4)]


IN_BF16 = True
OUT_BF16 = True


def build_bass():
    import concourse.mybir as mybir
    from concourse import bacc
    from concourse.tile import TileContext

    f32 = mybir.dt.float32
    in_dt = mybir.dt.bfloat16 if IN_BF16 else mybir.dt.float32r
    out_dt = mybir.dt.bfloat16 if OUT_BF16 else mybir.dt.float32
    e4 = 128 if IN_BF16 else 256
    segs = _w_segments(e4)
    wcols = sum(w for _, w, _ in segs)
    woff = np.cumsum([0] + [w for _, w, _ in segs])

    nc = bacc.Bacc(
        "TRN2",
        target_bir_lowering=False,
        debug=False,
        num_devices=NCORES,
    )
    x = nc.declare_dram_parameter("x", [G, 128, JBLK, 128], in_dt, isOutput=False)
    x0 = nc.declare_dram_parameter("x0", [128, 128], in_dt, isOutput=False)
    w = nc.declare_dram_parameter("w", [128, wcols], in_dt, isOutput=False)
    out = nc.declare_dram_parameter("out", [128, T_CORE], out_dt, isOutput=True)

    with TileContext(nc) as tc:
        with (
            tc.tile_pool(name="wp", bufs=1) as wp,
            tc.tile_pool(name="xp", bufs=8) as xp,
            tc.tile_pool(name="yp", bufs=6) as yp,
            tc.tile_pool(name="pp", bufs=8, space="PSUM") as pp,
        ):
            x0t = wp.tile([128, 128], in_dt)
            nc.sync.dma_start(out=x0t[:], in_=x0[:])
            wt = wp.tile([128, wcols], in_dt)
            nc.scalar.dma_start(out=wt[:], in_=w[:])

            # PE HAM warm-up: keep TensorE busy on throwaway matmuls while
            # the first input/weight DMAs are still in flight, so the clock
            # gate releases (1.2 -> 2.4 GHz) before the real matmuls start.
            warm_src = wp.tile([128, 128], in_dt)
            nc.gpsimd.memset(warm_src[:], 0.0)
            warm = pp.tile([128, TILE], f32, tag="pt")
            for _ in range(34):
                nc.tensor.matmul(
                    warm[:, :128], warm_src[:], warm_src[:], start=True, stop=True
                )

            prev_last = x0t[:]
            for g in range(G):
                xt = xp.tile([128, JBLK * 128], in_dt)
                nc.sync.dma_start(
                    out=xt[:], in_=x[g].rearrange("p j b -> p (j b)")
                )
                yt = yp.tile([128, S * TILE], out_dt)
                for s in range(S):
                    pt = pp.tile([128, TILE], f32)
                    for d in range(NBLK):
                        j = 4 * s + d - 1
                        lhsT = prev_last if j < 0 else xt[:, j * 128:(j + 1) * 128]
                        f0, width, _ = segs[d]
                        nc.tensor.matmul(
                            pt[:, f0:f0 + width],
                            lhsT,
                            wt[:, int(woff[d]):int(woff[d + 1])],
                            start=(d == 0),
                            stop=(d == NBLK - 1),
                        )
                    ysl = yt[:, s * TILE:(s + 1) * TILE]
                    nc.vector.tensor_copy(out=ysl, in_=pt[:])
                t0 = g * S * TILE
                for c0, c1 in ((0, 3), (3, 5), (5, 7)):
                    nc.scalar.dma_start(
                        out=out[:, t0 + c0 * TILE:t0 + c1 * TILE],
                        in_=yt[:, c0 * TILE:c1 * TILE],
                    )
                prev_last = xt[:, (JBLK - 1) * 128:JBLK * 128]
    nc.compile()
    return nc


def shard_inputs(wf):
    """wf: [128, 200000] f32 -> list of 8 in_maps."""
    if IN_BF16:
        import ml_dtypes
        np_in = ml_dtypes.bfloat16
    else:
        np_in = np.float32
    wmat = _toeplitz_weights(np_in, e4=128 if IN_BF16 else 256)
    X = np.zeros((B, 128 + T_PAD), dtype=np_in)
    X[:, 128:128 + T] = wf.astype(np_in)
    in_maps = []
    for c in range(NCORES):
        base = c * T_CORE
        sl = X[:, base:base + 128 + T_CORE]
        x0 = np.ascontiguousarray(sl[:, :128].T)
        body = sl[:, 128:]  # [b, T_CORE]
        xc = np.ascontiguousarray(
            body.reshape(B, G, JBLK, 128).transpose(1, 3, 2, 0)
        )
        in_maps.append({"x": xc, "x0": x0, "w": wmat})
    return in_maps


_CACHED = {}


def _run_with_retry(nc, in_maps):
    from concourse.bass_utils import run_bass_kernel_spmd

    last_err = None
    for attempt in range(3):
        try:
            return run_bass_kernel_spmd(nc, in_maps, core_ids=list(range(NCORES)))
        except Exception as e:  # transient device/runtime failures
            last_err = e
            try:
                import jax
                jax.clear_caches()
                jax.clear_backends()
            except Exception:
                pass
            import time
            time.sleep(3.0 * (attempt + 1))
    raise last_err


def kernel(waveform):
    wf = np.ascontiguousarray(np.asarray(waveform, dtype=np.float32))
    assert wf.shape == (B, T), wf.shape
    in_maps = shard_inputs(wf)
    if "nc" not in _CACHED:
        _CACHED["nc"] = build_bass()
    res = _run_with_retry(_CACHED["nc"], in_maps)
    y = np.concatenate(
        [np.asarray(res.results[c]["out"]) for c in range(NCORES)], axis=1
    )
    return np.ascontiguousarray(y[:, :T].astype(np.float32))
